# revision 22
# baseline (speedup 1.0000x reference)
"""Causal depthwise temporal conv (K=4) on 8 TRN2 NeuronCores.

Reference semantics (for x: [B, T, D], w: [K, D], b: [D]):
    out[bt, t, d] = sum_{j=0}^{K-1} x_pad[bt, t + j, d] * w[j, d] + b[d]
where x_pad is x left-padded with K-1 zeros along time.

Strategy (memory-bound problem; DMA is the floor at ~85us/core):
  - Tensor-parallel over channels: core m owns channels [m*512, (m+1)*512);
    depthwise conv => fully independent, no collectives.
  - fp16 on the wire (x in, out back) halves HBM traffic vs f32. The 2e-2
    correctness gate leaves ~40x headroom over the resulting ~5e-4 error.
  - Work is spread so no compute engine exceeds the DMA floor. DVE cannot
    run scalar_tensor_tensor chains above 1 elem/cycle on TRN2 (measured:
    fp16 STT = 1x mode, 4.8us per 4096-elem op), so the 4-tap FMA is
    restructured around PSUM accumulation:
      * ACT computes tap0 (+bias): a0 = w0*x + b      (SBUF fp16)
      * PE accumulates taps 1-3 as diagonal-stationary matmuls into
        psum (diag(w_j) @ x_shifted == w_j[ch]*x[ch, t+j])
      * DVE combines: out = psum + a0  (fp16, one 1x pass)
  - DMA ring discipline: loads on the SP HWDGE ring (2-row transfers),
    stores on the GPSIMD SWDGE ring. Separate rings, so stores that wait
    on compute can never head-of-line-block prefetch loads, and store
    triggers never block ACT's compute queue.
"""

import numpy as np

import concourse.bacc as bacc
import concourse.mybir as mybir
from concourse.tile import TileContext
from concourse import bass_utils

B = 4            # batch
T = 4096         # sequence length
D = 4096         # channels (width)
K = 4            # temporal taps
N_CORES = 8
D_SH = D // N_CORES          # 512 channels per core
P = 128                      # SBUF partitions
N_BLK = D_SH // P            # 4 channel blocks per core
TP = T + K - 1               # padded time length
RG = 2048                    # psum region width (4 banks)
MM = 512                     # matmul moving width (1 bank)


def _build(b=B, t=T, n_blk=N_BLK):
    nc = bacc.Bacc("TRN2")
    tp = t + K - 1
    f32 = mybir.dt.float32
    f16 = mybir.dt.float16
    mult, add = mybir.AluOpType.mult, mybir.AluOpType.add
    x = nc.dram_tensor("x", [n_blk, P, b, tp], f16, kind="ExternalInput")
    # per-block scalars: [:, 0]=w0 (ACT scale), [:, 1]=bias, [:, 2]=w1
    wt = nc.dram_tensor("wt", [n_blk, P, 4], f32, kind="ExternalInput")
    # per-block stationary diagonals for taps 1..3: [P, 3*P] fp16
    wd = nc.dram_tensor("wd", [n_blk, P, (K - 1) * P], f16,
                        kind="ExternalInput")
    out = nc.dram_tensor("out", [n_blk, P, b, t], f16, kind="ExternalOutput")
    ident_fn = mybir.ActivationFunctionType.Identity

    with TileContext(nc) as tc:
        with tc.tile_pool(name="xp", bufs=4) as xp, \
             tc.tile_pool(name="wp", bufs=2) as wp, \
             tc.tile_pool(name="op", bufs=6) as op, \
             tc.psum_pool(name="pp", bufs=2) as pp:
            # PE p-state warmup: dummy matmuls on a zeroed tile keep the
            # PE continuously busy through the preamble so the first real
            # matmuls run at 2.4GHz instead of the 1.2GHz ramp state.
            wu = wp.tile([P, MM], f16, tag="wu")
            nc.gpsimd.memset(wu[:], 0.0)
            wps = pp.tile([P, RG], f32, tag="ps")
            for _ in range(8):
                nc.tensor.matmul(wps[:, 0:MM], wu[:, 0:P], wu[:],
                                 start=True, stop=True)
            for blk in range(n_blk):
                wdt = wp.tile([P, (K - 1) * P], f16, tag="wd")
                wtt = wp.tile([P, 4], f32, tag="wt")
                if blk > 0:
                    # weight prefetch on the SP ring between row loads
                    # (NOT the ACT ring: there it FIFOs behind pending
                    # stores and stalls each block boundary by ~13us)
                    nc.sync.dma_start(wdt[:], wd[blk])
                    nc.sync.dma_start(wtt[:], wt[blk])
                for bb in range(0, b, 2):
                    first = blk == 0 and bb == 0
                    # one load covers two batch rows (fewer descriptors)
                    X2 = xp.tile([P, 2 * tp], f16, tag="x")
                    if first:
                        # split first load so the pipeline ramps in ~1us
                        # steps instead of waiting on a full 2MB transfer;
                        # chunks are disjoint (an overlap would chain
                        # region 2's reads onto chunk 2's completion)
                        cut1 = RG // 4 + K - 1
                        cut2 = RG // 2 + K - 1
                        cut3 = RG + K - 1
                        cut4 = 3 * RG // 2 + K - 1
                        nc.sync.dma_start(X2[:, 0:cut1],
                                          x[blk, :, bb, 0:cut1])
                        nc.sync.dma_start(wdt[:], wd[blk])
                        nc.sync.dma_start(wtt[:], wt[blk])
                        for lo, hi in ((cut1, cut2), (cut2, cut3),
                                       (cut3, cut4), (cut4, tp)):
                            nc.sync.dma_start(X2[:, lo:hi],
                                              x[blk, :, bb, lo:hi])
                        nc.sync.dma_start(X2[:, tp:],
                                          x[blk, :, bb + 1, :])
                    else:
                        nc.sync.dma_start(
                            X2[:], x[blk, :, bb:bb + 2, :])
                    for sub in range(2):
                        bbs = bb + sub
                        last = blk == n_blk - 1 and bbs == b - 1
                        X = X2[:, sub * tp:(sub + 1) * tp]
                        if first and sub == 0:
                            regions = [(0, RG // 4), (RG // 4, RG // 4),
                                       (RG // 2, RG // 2), (RG, RG // 2),
                                       (3 * RG // 2, RG // 2)]
                        elif last:
                            # small tail pieces drain the last stores fast
                            regions = [(0, RG), (RG, RG // 2),
                                       (RG + RG // 2, RG // 4),
                                       (RG + 3 * RG // 4, RG // 4)]
                        else:
                            regions = [(0, RG), (RG, RG)]
                        O = op.tile([P, t], f16, tag="o")
                        for c, rg in regions:
                            # tap 0 (+bias) on ACT, independent of the
                            # psum chain (no cross-engine psum RMW race).
                            a0 = op.tile([P, RG], f16, tag="a0")
                            nc.scalar.activation(a0[:, :rg], X[:, c:c + rg],
                                                 ident_fn,
                                                 bias=wtt[:, 1:2],
                                                 scale=wtt[:, 0:1])
                            acc, j0 = a0, 1
                            # remaining taps accumulate in psum via diag
                            # matmuls; PE owns the banks from reset.
                            ps = pp.tile([P, RG], f32, tag="ps")
                            for j in range(j0, K):
                                dg = wdt[:, (j - 1) * P:j * P]
                                for k in range(0, rg, MM):
                                    nc.tensor.matmul(
                                        ps[:, k:k + MM], dg,
                                        X[:, c + j + k:c + j + k + MM],
                                        start=(j == j0), stop=(j == K - 1))
                            # combine: out = psum(PE taps) + acc
                            nc.vector.tensor_tensor(
                                O[:, c:c + rg], ps[:, :rg], acc[:, :rg],
                                add)
                            if last:
                                # loads are done by now; the idle SP ring
                                # drains the tail faster than SWDGE
                                nc.sync.dma_start(
                                    out[blk, :, bbs, c:c + rg],
                                    O[:, c:c + rg])
                        if not last:
                            # stores ride the GPSIMD SWDGE ring: their own
                            # sequencer, so pending stores never FIFO-block
                            # loads (SP ring) or activations (ACT queue)
                            nc.gpsimd.dma_start(out[blk, :, bbs, :], O[:])
    nc.compile()
    return nc


def _prepare(x, w, b):
    x = np.asarray(x, dtype=np.float32)
    w = np.asarray(w, dtype=np.float32)
    b = np.asarray(b, dtype=np.float32)
    # channel-major, left zero-padded time: [D, B, TP], fp16 on the wire
    xp = np.zeros((D, B, TP), dtype=np.float16)
    xp[:, :, K - 1:] = x.transpose(2, 0, 1)
    # per-channel scalars: w0 (ACT scale), bias, w1 (GPSIMD tap1)
    wbt = np.stack([w[0], b, w[1], np.zeros_like(b)], axis=1).astype(np.float32)
    # stationary diagonals: wdall[d, (j-1)*P + m] = w[j, d] iff m == d%P
    wdall = np.zeros((D, (K - 1) * P), dtype=np.float16)
    for j in range(1, K):
        cols = (j - 1) * P + (np.arange(D) % P)
        wdall[np.arange(D), cols] = w[j].astype(np.float16)
    in_maps = []
    for m in range(N_CORES):
        sl = slice(m * D_SH, (m + 1) * D_SH)
        in_maps.append({
            "x": np.ascontiguousarray(xp[sl]).reshape(N_BLK, P, B, TP),
            "wt": np.ascontiguousarray(wbt[sl]).reshape(N_BLK, P, 4),
            "wd": np.ascontiguousarray(wdall[sl]).reshape(N_BLK, P, (K - 1) * P),
        })
    return in_maps


def _collect(results):
    out = np.empty((B, T, D), dtype=np.float32)
    for m in range(N_CORES):
        o = np.asarray(results[m]["out"]).astype(np.float32).reshape(D_SH, B, T)
        out[:, :, m * D_SH:(m + 1) * D_SH] = o.transpose(1, 2, 0)
    return out


def _run(in_maps, trace=False, **kwargs):
    nc = _build()
    return bass_utils.run_bass_kernel_spmd(
        nc, in_maps, core_ids=list(range(N_CORES)), trace=trace, **kwargs)


def kernel(x, w, b):
    in_maps = _prepare(x, w, b)
    try:
        res = _run(in_maps)
    except Exception:
        # Transient NRT device errors have been observed on a cold first
        # execute; one retry (fresh compile dir) clears them.
        res = _run(in_maps)
    return _collect(res.results)


# revision 24
# speedup vs baseline: 1.0125x; 1.0125x over previous
"""Causal depthwise temporal conv (K=4) on 8 TRN2 NeuronCores.

Reference semantics (for x: [B, T, D], w: [K, D], b: [D]):
    out[bt, t, d] = sum_{j=0}^{K-1} x_pad[bt, t + j, d] * w[j, d] + b[d]
where x_pad is x left-padded with K-1 zeros along time.

Strategy (memory-bound problem; DMA is the floor at ~85us/core):
  - Tensor-parallel over channels: core m owns channels [m*512, (m+1)*512);
    depthwise conv => fully independent, no collectives.
  - fp16 on the wire (x in, out back) halves HBM traffic vs f32. The 2e-2
    correctness gate leaves ~40x headroom over the resulting ~5e-4 error.
  - Work is spread so no compute engine exceeds the DMA floor. DVE cannot
    run scalar_tensor_tensor chains above 1 elem/cycle on TRN2 (measured:
    fp16 STT = 1x mode, 4.8us per 4096-elem op), so the 4-tap FMA is
    restructured around PSUM accumulation:
      * ACT computes tap0 (+bias): a0 = w0*x + b      (SBUF fp16)
      * PE accumulates taps 1-3 as diagonal-stationary matmuls into
        psum (diag(w_j) @ x_shifted == w_j[ch]*x[ch, t+j])
      * DVE combines: out = psum + a0  (fp16, one 1x pass)
  - DMA ring discipline: loads on the SP HWDGE ring (2-row transfers),
    stores on the GPSIMD SWDGE ring. Separate rings, so stores that wait
    on compute can never head-of-line-block prefetch loads, and store
    triggers never block ACT's compute queue.
"""

import numpy as np

import concourse.bacc as bacc
import concourse.mybir as mybir
from concourse.tile import TileContext
from concourse import bass_utils

B = 4            # batch
T = 4096         # sequence length
D = 4096         # channels (width)
K = 4            # temporal taps
N_CORES = 8
D_SH = D // N_CORES          # 512 channels per core
P = 128                      # SBUF partitions
N_BLK = D_SH // P            # 4 channel blocks per core
TP = T + K - 1               # padded time length
RG = 2048                    # psum region width (4 banks)
MM = 512                     # matmul moving width (1 bank)


def _build(b=B, t=T, n_blk=N_BLK):
    nc = bacc.Bacc("TRN2")
    tp = t + K - 1
    f32 = mybir.dt.float32
    f16 = mybir.dt.float16
    mult, add = mybir.AluOpType.mult, mybir.AluOpType.add
    x = nc.dram_tensor("x", [n_blk, P, b, tp], f16, kind="ExternalInput")
    # per-block scalars: [:, 0]=w0 (ACT scale), [:, 1]=bias, [:, 2]=w1
    wt = nc.dram_tensor("wt", [n_blk, P, 4], f32, kind="ExternalInput")
    # per-block stationary diagonals for taps 1..3: [P, 3*P] fp16
    wd = nc.dram_tensor("wd", [n_blk, P, (K - 1) * P], f16,
                        kind="ExternalInput")
    out = nc.dram_tensor("out", [n_blk, P, b, t], f16, kind="ExternalOutput")
    ident_fn = mybir.ActivationFunctionType.Identity

    with TileContext(nc) as tc:
        with tc.tile_pool(name="xp", bufs=4) as xp, \
             tc.tile_pool(name="wp", bufs=2) as wp, \
             tc.tile_pool(name="op", bufs=6) as op, \
             tc.psum_pool(name="pp", bufs=2) as pp:
            # PE p-state warmup: dummy matmuls on a zeroed tile keep the
            # PE continuously busy through the preamble so the first real
            # matmuls run at 2.4GHz instead of the 1.2GHz ramp state.
            wu = wp.tile([P, MM], f16, tag="wu")
            nc.gpsimd.memset(wu[:], 0.0)
            wps = pp.tile([P, RG], f32, tag="ps")
            for _ in range(8):
                nc.tensor.matmul(wps[:, 0:MM], wu[:, 0:P], wu[:],
                                 start=True, stop=True)
            for blk in range(n_blk):
                wdt = wp.tile([P, (K - 1) * P], f16, tag="wd")
                wtt = wp.tile([P, 4], f32, tag="wt")
                if blk > 0:
                    # weight prefetch on the SP ring between row loads
                    # (NOT the ACT ring: there it FIFOs behind pending
                    # stores and stalls each block boundary by ~13us)
                    nc.sync.dma_start(wdt[:], wd[blk])
                    nc.sync.dma_start(wtt[:], wt[blk])
                for bb in range(0, b, 2):
                    first = blk == 0 and bb == 0
                    # one load covers two batch rows (fewer descriptors)
                    X2 = xp.tile([P, 2 * tp], f16, tag="x")
                    if first:
                        # split first load so the pipeline ramps in ~1us
                        # steps instead of waiting on a full 2MB transfer;
                        # chunks are disjoint (an overlap would chain
                        # region 2's reads onto chunk 2's completion)
                        cut1 = RG // 4 + K - 1
                        cut2 = RG // 2 + K - 1
                        cut3 = RG + K - 1
                        nc.sync.dma_start(X2[:, 0:cut1],
                                          x[blk, :, bb, 0:cut1])
                        nc.sync.dma_start(wdt[:], wd[blk])
                        nc.sync.dma_start(wtt[:], wt[blk])
                        for lo, hi in ((cut1, cut2), (cut2, cut3),
                                       (cut3, tp)):
                            nc.sync.dma_start(X2[:, lo:hi],
                                              x[blk, :, bb, lo:hi])
                        nc.sync.dma_start(X2[:, tp:],
                                          x[blk, :, bb + 1, :])
                    else:
                        nc.sync.dma_start(
                            X2[:], x[blk, :, bb:bb + 2, :])
                    for sub in range(2):
                        bbs = bb + sub
                        last = blk == n_blk - 1 and bbs == b - 1
                        X = X2[:, sub * tp:(sub + 1) * tp]
                        if first and sub == 0:
                            regions = [(0, RG // 4), (RG // 4, RG // 4),
                                       (RG // 2, RG // 2), (RG, RG)]
                        elif last:
                            # small tail pieces drain the last stores fast
                            regions = [(0, RG), (RG, RG // 2),
                                       (RG + RG // 2, RG // 4),
                                       (RG + 3 * RG // 4, RG // 4)]
                        else:
                            regions = [(0, RG), (RG, RG)]
                        O = op.tile([P, t], f16, tag="o")
                        for c, rg in regions:
                            # tap 0 (+bias) on ACT, independent of the
                            # psum chain (no cross-engine psum RMW race).
                            a0 = op.tile([P, RG], f16, tag="a0")
                            nc.scalar.activation(a0[:, :rg], X[:, c:c + rg],
                                                 ident_fn,
                                                 bias=wtt[:, 1:2],
                                                 scale=wtt[:, 0:1])
                            acc, j0 = a0, 1
                            # remaining taps accumulate in psum via diag
                            # matmuls; PE owns the banks from reset.
                            ps = pp.tile([P, RG], f32, tag="ps")
                            for j in range(j0, K):
                                dg = wdt[:, (j - 1) * P:j * P]
                                for k in range(0, rg, MM):
                                    nc.tensor.matmul(
                                        ps[:, k:k + MM], dg,
                                        X[:, c + j + k:c + j + k + MM],
                                        start=(j == j0), stop=(j == K - 1))
                            # combine: out = psum(PE taps) + acc
                            nc.vector.tensor_tensor(
                                O[:, c:c + rg], ps[:, :rg], acc[:, :rg],
                                add)
                            if last:
                                # loads are done by now; the idle SP ring
                                # drains the tail faster than SWDGE
                                nc.sync.dma_start(
                                    out[blk, :, bbs, c:c + rg],
                                    O[:, c:c + rg])
                        if not last:
                            # stores ride the GPSIMD SWDGE ring: their own
                            # sequencer, so pending stores never FIFO-block
                            # loads (SP ring) or activations (ACT queue)
                            nc.gpsimd.dma_start(out[blk, :, bbs, :], O[:])
    nc.compile()
    return nc


def _prepare(x, w, b):
    x = np.asarray(x, dtype=np.float32)
    w = np.asarray(w, dtype=np.float32)
    b = np.asarray(b, dtype=np.float32)
    # channel-major, left zero-padded time: [D, B, TP], fp16 on the wire
    xp = np.zeros((D, B, TP), dtype=np.float16)
    xp[:, :, K - 1:] = x.transpose(2, 0, 1)
    # per-channel scalars: w0 (ACT scale), bias, w1 (GPSIMD tap1)
    wbt = np.stack([w[0], b, w[1], np.zeros_like(b)], axis=1).astype(np.float32)
    # stationary diagonals: wdall[d, (j-1)*P + m] = w[j, d] iff m == d%P
    wdall = np.zeros((D, (K - 1) * P), dtype=np.float16)
    for j in range(1, K):
        cols = (j - 1) * P + (np.arange(D) % P)
        wdall[np.arange(D), cols] = w[j].astype(np.float16)
    in_maps = []
    for m in range(N_CORES):
        sl = slice(m * D_SH, (m + 1) * D_SH)
        in_maps.append({
            "x": np.ascontiguousarray(xp[sl]).reshape(N_BLK, P, B, TP),
            "wt": np.ascontiguousarray(wbt[sl]).reshape(N_BLK, P, 4),
            "wd": np.ascontiguousarray(wdall[sl]).reshape(N_BLK, P, (K - 1) * P),
        })
    return in_maps


def _collect(results):
    out = np.empty((B, T, D), dtype=np.float32)
    for m in range(N_CORES):
        o = np.asarray(results[m]["out"]).astype(np.float32).reshape(D_SH, B, T)
        out[:, :, m * D_SH:(m + 1) * D_SH] = o.transpose(1, 2, 0)
    return out


def _run(in_maps, trace=False, **kwargs):
    nc = _build()
    return bass_utils.run_bass_kernel_spmd(
        nc, in_maps, core_ids=list(range(N_CORES)), trace=trace, **kwargs)


def kernel(x, w, b):
    in_maps = _prepare(x, w, b)
    try:
        res = _run(in_maps)
    except Exception:
        # Transient NRT device errors have been observed on a cold first
        # execute; one retry (fresh compile dir) clears them.
        res = _run(in_maps)
    return _collect(res.results)
